# revision 11
# baseline (speedup 1.0000x reference)
"""GAT (3-layer, PyG-style) forward on 8 Trainium2 NeuronCores via Bass/Tile.

Strategy (dst-partitioned edges + AllGathered projection table):
  - Nodes are split into 8 contiguous shards (6250 each). Each core owns the
    edges whose *destination* lies in its shard (plus self loops), sorted by
    destination.
  - Per layer: each core projects its node shard (h @ [W | W~src | W~dst]) so
    every table row is [xp (d_out) | a_src (H) | a_dst (H)]; shards are
    AllGathered so each core holds the full projection table in local HBM.
  - Edge phase: edges are grouped by 128-node destination windows, padded to a
    uniform number of 128-edge tiles per window (uniform across cores: SPMD
    needs one program). Per window: one indirect DMA gathers all source rows,
    per-edge logits/softmax numerators are computed on DVE/ACT, and a 0/1
    selection matrix S[e,v] = (dst_e == v) turns the segment scatter-add into
    PE matmuls accumulating in PSUM (numerator and denominator together).
  - Softmax uses exp without max subtraction (logits are O(1) here; exact same
    math as the reference up to fp rounding).
  - Layer output windows are normalized, biased, GELU'd, transposed (PE) and
    written back as h^T for the next layer's projection.
  - After layer 3: global mean pool via one-hot(batch) matmuls accumulated in
    PSUM over windows, AllReduce of [64, 65] partials, divide, done.
"""

import math
import numpy as np

import concourse.bass as bass
import concourse.bacc as bacc
import concourse.mybir as mybir
import concourse.tile as tile
from concourse.masks import make_identity

F32 = mybir.dt.float32
F32R = mybir.dt.float32r
BF16 = mybir.dt.bfloat16
I32 = mybir.dt.int32
I16 = mybir.dt.int16


class GATCfg:
    def __init__(self, N, E, B, Fin, layers, NC=8):
        # layers: list of dicts with H, C, concat
        self.N, self.E, self.B, self.Fin, self.NC = N, E, B, Fin, NC
        assert N % NC == 0
        self.NPC = N // NC
        self.NW = math.ceil(self.NPC / 128)
        self.NPCp = self.NW * 128
        self.layers = []
        d_in = Fin
        for l in layers:
            H, C, concat = l["H"], l["C"], l["concat"]
            d_out = H * C
            self.layers.append(
                dict(d_in=d_in, H=H, C=C, d_out=d_out, concat=concat,
                     R=d_out + 2 * H, db=(d_out if concat else C), ROW=d_out + 2 * H)
            )
            d_in = d_out if concat else C


REAL_CFG = GATCfg(
    N=50000, E=400000, B=64, Fin=128,
    layers=[dict(H=4, C=16, concat=True),
            dict(H=4, C=64, concat=True),
            dict(H=4, C=64, concat=False)],
)


# ---------------------------------------------------------------- host prep
def _host_prep(cfg, x, edge_index, batch, Ws, As, Ad, Bs):
    """Returns (in_maps, T_w). Ws/As/Ad/Bs: per-layer weight lists."""
    N, NC, NPC, NPCp, NW = cfg.N, cfg.NC, cfg.NPC, cfg.NPCp, cfg.NW
    src = np.concatenate([edge_index[0], np.arange(N, dtype=np.int64)])
    dst = np.concatenate([edge_index[1], np.arange(N, dtype=np.int64)])
    core_of = dst // NPC

    src_pad = (src // NPC) * NPCp + src % NPC
    win_global = (dst % NPC) // 128 + core_of * NW
    cnts = np.bincount(win_global, minlength=NC * NW)
    T_w = int(np.ceil(cnts.max() / 128))

    per_core = []
    for c in range(NC):
        sel = np.nonzero(core_of == c)[0]
        dloc = (dst[sel] - c * NPC).astype(np.int64)
        sp = src_pad[sel]
        win = dloc // 128
        order = np.argsort(win, kind="stable")
        sel, dloc, sp, win = sel[order], dloc[order], sp[order], win[order]
        wstart = np.searchsorted(win, np.arange(NW))
        slot = np.arange(len(sel)) - wstart[win]
        jj, pp = slot // 128, slot % 128

        esrc = np.zeros((NW, 128, T_w), np.int32)
        edst = np.zeros((NW, 128, T_w), np.int32)
        edrel = np.full((NW, 128, T_w), -1.0, np.float32)
        esrc[win, pp, jj] = sp.astype(np.int32)
        edst[win, pp, jj] = ((dst[sel] // NPC) * NPCp + dst[sel] % NPC).astype(np.int32)
        edrel[win, pp, jj] = (dloc - win * 128).astype(np.float32)

        batchf = np.full((NW, 128, 1), -1.0, np.float32)
        bloc = batch[c * NPC:(c + 1) * NPC].astype(np.float32)
        bf = np.full(NPCp, -1.0, np.float32)
        bf[:NPC] = bloc
        batchf[:, :, 0] = bf.reshape(NW, 128)

        xT = np.zeros((cfg.Fin, NPCp), np.float32)
        xT[:, :NPC] = x[c * NPC:(c + 1) * NPC].T

        m = dict(xT=xT, esrc=esrc, edst=edst, edrel=edrel, batchf=batchf)
        for li, (W, a_s, a_d) in enumerate(zip(Ws, As, Ad)):
            L = cfg.layers[li]
            H, C, d_in, d_out = L["H"], L["C"], L["d_in"], L["d_out"]
            Wr = W.reshape(d_in, H, C)
            Wts = np.einsum("khc,hc->kh", Wr, a_s).astype(np.float32)
            Wtd = np.einsum("khc,hc->kh", Wr, a_d).astype(np.float32)
            m[f"waug{li}"] = np.concatenate([W, Wts, Wtd], axis=1).astype(np.float32)
            m[f"bias{li}"] = np.broadcast_to(Bs[li], (128, L["db"])).astype(np.float32).copy()
        per_core.append(m)
    return per_core, T_w


# ---------------------------------------------------------------- program
def _build_program(cfg, T_w):
    NC, NPCp, NW, B = cfg.NC, cfg.NPCp, cfg.NW, cfg.B
    NL = len(cfg.layers)
    nc = bacc.Bacc("TRN2", target_bir_lowering=False, debug=False,
                   enable_asserts=False, num_devices=cfg.NC)

    # ---- I/O
    xT_p = nc.declare_dram_parameter("xT", [cfg.Fin, NPCp], F32, isOutput=False)
    esrc_p = nc.declare_dram_parameter("esrc", [NW, 128, T_w], I32, isOutput=False)
    edst_p = nc.declare_dram_parameter("edst", [NW, 128, T_w], I32, isOutput=False)
    edrel_p = nc.declare_dram_parameter("edrel", [NW, 128, T_w], F32, isOutput=False)
    batchf_p = nc.declare_dram_parameter("batchf", [NW, 128, 1], F32, isOutput=False)
    waug_p, bias_p = [], []
    for li, L in enumerate(cfg.layers):
        waug_p.append(nc.declare_dram_parameter(f"waug{li}", [L["d_in"], L["R"]], F32, isOutput=False))
        bias_p.append(nc.declare_dram_parameter(f"bias{li}", [128, L["db"]], F32, isOutput=False))
    out_p = nc.declare_dram_parameter("out", [B, cfg.layers[-1]["C"]], F32, isOutput=True)

    # ---- internal DRAM
    tabloc = [nc.dram_tensor(f"tabloc{li}", [NPCp, L["ROW"]], BF16)
              for li, L in enumerate(cfg.layers)]
    tabfull = [nc.dram_tensor(f"tabfull{li}", [NC * NPCp, L["ROW"]], BF16, addr_space="Shared")
               for li, L in enumerate(cfg.layers)]

    hT = [None]
    for li in range(1, NL):
        hT.append(nc.dram_tensor(f"hT{li}", [cfg.layers[li]["d_in"], NPCp], F32))
    poolpart = nc.dram_tensor("poolpart", [B, cfg.layers[-1]["C"] + 1], F32)
    poolsum = nc.dram_tensor("poolsum", [B, cfg.layers[-1]["C"] + 1], F32, addr_space="Shared")

    rg = [list(range(NC))]

    with tile.TileContext(nc) as tc:
        with (
            tc.tile_pool(name="const", bufs=1) as constp,
            tc.tile_pool(name="wts", bufs=1) as wtsp,
            tc.tile_pool(name="proj", bufs=3) as projp,
            tc.tile_pool(name="edge", bufs=2) as edgep,
            tc.tile_pool(name="fin", bufs=2) as finp,
            tc.tile_pool(name="psmm", bufs=2, space="PSUM") as psmm,
            tc.tile_pool(name="pswin", bufs=2, space="PSUM") as pswin,
            tc.tile_pool(name="pstr", bufs=2, space="PSUM") as pstr,
            tc.tile_pool(name="pspool", bufs=1, space="PSUM") as pspool,
        ):
            # constants
            iota_f = constp.tile([128, 128], F32)
            nc.gpsimd.iota(iota_f[:], pattern=[[1, 128]], base=0,
                           channel_multiplier=0, allow_small_or_imprecise_dtypes=True)
            ident = constp.tile([128, 128], F32)
            make_identity(nc, ident[:])
            ones = constp.tile([128, 1], F32)
            nc.vector.memset(ones[:], 1.0)

            # weights / biases resident in SBUF
            waug_sb, bias_sb = [], []
            for li, L in enumerate(cfg.layers):
                chunks = []
                d_in = L["d_in"]
                for k in range(0, d_in, 128):
                    kc = min(128, d_in - k)
                    wt = wtsp.tile([kc, L["R"]], F32, tag=f"w{li}_{k}")
                    nc.sync.dma_start(out=wt[:], in_=waug_p[li][k:k + kc, :])
                    chunks.append(wt)
                waug_sb.append(chunks)
                bt = wtsp.tile([128, L["db"]], F32, tag=f"b{li}")
                nc.sync.dma_start(out=bt[:], in_=bias_p[li][:, :])
                bias_sb.append(bt)

            pool_ps = pspool.tile([B, cfg.layers[-1]["C"] + 1], F32)

            for li, L in enumerate(cfg.layers):
                d_in, d_out, H, C, R = L["d_in"], L["d_out"], L["H"], L["C"], L["R"]
                R2 = d_out + H
                concat = L["concat"]

                # ---------------- phase A: projection + table + AllGather
                for m in range(NW):
                    ps = psmm.tile([128, R], F32)
                    nk = (d_in + 127) // 128
                    for ki, k in enumerate(range(0, d_in, 128)):
                        kc = min(128, d_in - k)
                        lh = projp.tile([kc, 128], F32, tag="lh")
                        if li == 0:
                            nc.sync.dma_start(out=lh[:], in_=xT_p[k:k + kc, m * 128:(m + 1) * 128])
                        else:
                            nc.sync.dma_start(out=lh[:], in_=hT[li][k:k + kc, m * 128:(m + 1) * 128])
                        nc.tensor.matmul(out=ps[:], lhsT=lh[:], rhs=waug_sb[li][ki][:],
                                         start=(ki == 0), stop=(ki == nk - 1))
                    tabt = projp.tile([128, L["ROW"]], BF16, tag="tabt")
                    nc.scalar.activation(out=tabt[:], in_=ps[:],
                                         func=mybir.ActivationFunctionType.Copy)
                    nc.sync.dma_start(out=tabloc[li][m * 128:(m + 1) * 128, :], in_=tabt[:])

                nc.gpsimd.collective_compute(
                    "AllGather", mybir.AluOpType.bypass, replica_groups=rg,
                    ins=[tabloc[li][:, :]], outs=[tabfull[li][:, :]],
                )

                # ---------------- phase B: edges, one 128-node window at a time
                ROW = L["ROW"]
                for w in range(NW):
                    sidx = edgep.tile([128, T_w], I32, tag="sidx")
                    nc.sync.dma_start(out=sidx[:], in_=esrc_p[w, :, :])
                    didx = edgep.tile([128, T_w], I32, tag="didx")
                    nc.sync.dma_start(out=didx[:], in_=edst_p[w, :, :])
                    drel = edgep.tile([128, T_w], F32, tag="drel")
                    nc.sync.dma_start(out=drel[:], in_=edrel_p[w, :, :])

                    G = edgep.tile([128, T_w, ROW], BF16, tag="G")
                    G2 = edgep.tile([128, T_w, H], BF16, tag="G2")
                    for j in range(T_w):
                        nc.gpsimd.indirect_dma_start(
                            out=G[:, j, :], out_offset=None, in_=tabfull[li][:, :],
                            in_offset=bass.IndirectOffsetOnAxis(ap=sidx[:, j:j + 1], axis=0),
                        )
                        nc.gpsimd.indirect_dma_start(
                            out=G2[:, j, :], out_offset=None, in_=tabfull[li][:, :],
                            in_offset=bass.IndirectOffsetOnAxis(ap=didx[:, j:j + 1], axis=0),
                            element_offset=d_out + H,
                        )

                    # S[e, v] = (dst_rel[e] == v), 0/1 in f32
                    S = edgep.tile([128, T_w, 128], BF16, tag="S")
                    nc.vector.tensor_tensor(
                        out=S[:, :, :],
                        in0=drel[:, :, None].to_broadcast([128, T_w, 128]),
                        in1=iota_f[:, None, :].to_broadcast([128, T_w, 128]),
                        op=mybir.AluOpType.is_equal,
                    )

                    # logits -> p = exp(leaky_relu(a_src[src] + a_dst[dst]))
                    z = edgep.tile([128, T_w, H], F32, tag="z")
                    nc.vector.tensor_add(out=z[:, :, :], in0=G[:, :, d_out:d_out + H],
                                         in1=G2[:, :, :])
                    zs = edgep.tile([128, T_w, H], F32, tag="zs")
                    nc.scalar.activation(out=zs[:, :, :], in_=z[:, :, :],
                                         func=mybir.ActivationFunctionType.Copy, scale=0.2)
                    zm = edgep.tile([128, T_w, H], F32, tag="zm")
                    nc.vector.tensor_max(out=zm[:, :, :], in0=z[:, :, :], in1=zs[:, :, :])
                    MT = edgep.tile([128, T_w, R2], BF16, tag="MT")
                    pf = edgep.tile([128, T_w, H], F32, tag="pf")
                    nc.scalar.activation(out=pf[:, :, :], in_=zm[:, :, :],
                                         func=mybir.ActivationFunctionType.Exp)
                    nc.vector.tensor_copy(out=MT[:, :, d_out:], in_=pf[:, :, :])
                    # M[e, h*C:(h+1)C] = p[e,h] * xp[src_e, h, :]
                    for h in range(H):
                        nc.vector.tensor_mul(
                            out=MT[:, :, h * C:(h + 1) * C],
                            in0=G[:, :, h * C:(h + 1) * C],
                            in1=MT[:, :, d_out + h:d_out + h + 1].to_broadcast([128, T_w, C]),
                        )

                    ps_w = pswin.tile([128, R2], F32)
                    for j in range(T_w):
                        nc.tensor.matmul(out=ps_w[:], lhsT=S[:, j, :], rhs=MT[:, j, :],
                                         start=(j == 0), stop=(j == T_w - 1))

                    # normalize: attn[:, hC:(h+1)C] = num / (den + eps)
                    den = finp.tile([128, H], F32, tag="den")
                    nc.vector.tensor_scalar_add(out=den[:], in0=ps_w[:, d_out:], scalar1=1e-16)
                    rcp = finp.tile([128, H], F32, tag="rcp")
                    nc.vector.reciprocal(out=rcp[:], in_=den[:])
                    attn = finp.tile([128, d_out], F32, tag="attn")
                    for h in range(H):
                        nc.scalar.activation(out=attn[:, h * C:(h + 1) * C],
                                             in_=ps_w[:, h * C:(h + 1) * C],
                                             func=mybir.ActivationFunctionType.Copy,
                                             scale=rcp[:, h:h + 1])

                    hn = finp.tile([128, L["db"] + (0 if concat else 1)], F32, tag="hn")
                    if concat:
                        hp = finp.tile([128, d_out], F32, tag="hp")
                        nc.vector.tensor_add(out=hp[:], in0=attn[:], in1=bias_sb[li][:])
                        nc.scalar.activation(out=hn[:], in_=hp[:],
                                             func=mybir.ActivationFunctionType.Gelu)
                    else:
                        hm = finp.tile([128, C], F32, tag="hm")
                        nc.vector.tensor_add(out=hm[:], in0=attn[:, 0:C], in1=attn[:, C:2 * C])
                        for h in range(2, H):
                            nc.vector.tensor_add(out=hm[:], in0=hm[:], in1=attn[:, h * C:(h + 1) * C])
                        hb = finp.tile([128, C], F32, tag="hb")
                        nc.vector.tensor_scalar(out=hb[:], in0=hm[:], scalar1=1.0 / H,
                                                scalar2=None, op0=mybir.AluOpType.mult)
                        hp2 = finp.tile([128, C], F32, tag="hp2")
                        nc.vector.tensor_add(out=hp2[:], in0=hb[:], in1=bias_sb[li][:])
                        nc.scalar.activation(out=hn[:, :C], in_=hp2[:],
                                             func=mybir.ActivationFunctionType.Gelu)
                        nc.vector.memset(hn[:, C:], 1.0)

                    if li < NL - 1:
                        # store h^T for the next projection
                        dn = L["db"]
                        for k in range(0, dn, 128):
                            kc = min(128, dn - k)
                            pt = pstr.tile([kc, 128], F32, tag="pt")
                            nc.tensor.transpose(out=pt[:], in_=hn[:, k:k + kc], identity=ident[:])
                            ht_sb = finp.tile([kc, 128], F32, tag="htsb")
                            nc.scalar.activation(out=ht_sb[:], in_=pt[:],
                                                 func=mybir.ActivationFunctionType.Copy)
                            nc.sync.dma_start(out=hT[li + 1][k:k + kc, w * 128:(w + 1) * 128],
                                              in_=ht_sb[:])
                    else:
                        # global mean pool partials: one-hot(batch) matmuls
                        bf = edgep.tile([128, 1], F32, tag="bf")
                        nc.sync.dma_start(out=bf[:], in_=batchf_p[w, :, :])
                        bsel = finp.tile([128, B], F32, tag="bsel")
                        nc.vector.tensor_tensor(
                            out=bsel[:], in0=bf[:, :1].to_broadcast([128, B]),
                            in1=iota_f[:, :B], op=mybir.AluOpType.is_equal,
                        )
                        nc.tensor.matmul(out=pool_ps[:], lhsT=bsel[:], rhs=hn[:],
                                         start=(w == 0), stop=(w == NW - 1))

            # ---------------- final pooling: AllReduce partials, divide
            C = cfg.layers[-1]["C"]
            pps = finp.tile([B, C + 1], F32, tag="pps")
            nc.scalar.activation(out=pps[:], in_=pool_ps[:],
                                 func=mybir.ActivationFunctionType.Copy)
            nc.sync.dma_start(out=poolpart[:, :], in_=pps[:])
            nc.gpsimd.collective_compute(
                "AllReduce", mybir.AluOpType.add, replica_groups=rg,
                ins=[poolpart[:, :]], outs=[poolsum[:, :]],
            )
            pl = finp.tile([B, C + 1], F32, tag="pl")
            nc.sync.dma_start(out=pl[:], in_=poolsum[:, :])
            cnt = finp.tile([B, 1], F32, tag="cnt")
            nc.vector.tensor_scalar_max(out=cnt[:], in0=pl[:, C:C + 1], scalar1=1.0)
            rc = finp.tile([B, 1], F32, tag="rc")
            nc.vector.reciprocal(out=rc[:], in_=cnt[:])
            om = finp.tile([B, C], F32, tag="om")
            nc.vector.tensor_mul(out=om[:], in0=pl[:, :C],
                                 in1=rc[:, :1].to_broadcast([B, C]))
            nc.sync.dma_start(out=out_p[:, :], in_=om[:])

    nc.finalize()
    return nc


# ---------------------------------------------------------------- entry
def _prep_and_build(cfg, x, edge_index, batch, Ws, As, Ad, Bs):
    in_maps, T_w = _host_prep(cfg, np.asarray(x), np.asarray(edge_index),
                              np.asarray(batch), Ws, As, Ad, Bs)
    nc = _build_program(cfg, T_w)
    return nc, in_maps


def kernel(x, edge_index, batch, W0, as0, ad0, b0, W1, as1, ad1, b1, W2, as2, ad2, b2):
    from concourse.bass_utils import run_bass_kernel_spmd

    cfg = REAL_CFG
    nc, in_maps = _prep_and_build(
        cfg, x, edge_index, batch,
        [np.asarray(W0), np.asarray(W1), np.asarray(W2)],
        [np.asarray(as0), np.asarray(as1), np.asarray(as2)],
        [np.asarray(ad0), np.asarray(ad1), np.asarray(ad2)],
        [np.asarray(b0), np.asarray(b1), np.asarray(b2)],
    )
    res = run_bass_kernel_spmd(nc, in_maps, list(range(cfg.NC)))
    return np.asarray(res.results[0]["out"], dtype=np.float32)


# revision 12
# speedup vs baseline: 1.7866x; 1.7866x over previous
"""GAT (3-layer, PyG-style) forward on 8 Trainium2 NeuronCores via Bass/Tile.

Strategy (dst-partitioned edges + AllGathered projection table):
  - Nodes are split into 8 contiguous shards (6250 each). Each core owns the
    edges whose *destination* lies in its shard (plus self loops), sorted by
    destination.
  - Per layer: each core projects its node shard (h @ [W | W~src | W~dst]) so
    every table row is [xp (d_out) | a_src (H) | a_dst (H)]; shards are
    AllGathered so each core holds the full projection table in local HBM.
  - Edge phase: edges are grouped by 128-node destination windows, padded to a
    uniform number of 128-edge tiles per window (uniform across cores: SPMD
    needs one program). Per window: one indirect DMA gathers all source rows,
    per-edge logits/softmax numerators are computed on DVE/ACT, and a 0/1
    selection matrix S[e,v] = (dst_e == v) turns the segment scatter-add into
    PE matmuls accumulating in PSUM (numerator and denominator together).
  - Softmax uses exp without max subtraction (logits are O(1) here; exact same
    math as the reference up to fp rounding).
  - Layer output windows are normalized, biased, GELU'd, transposed (PE) and
    written back as h^T for the next layer's projection.
  - After layer 3: global mean pool via one-hot(batch) matmuls accumulated in
    PSUM over windows, AllReduce of [64, 65] partials, divide, done.
"""

import math
import numpy as np

import concourse.bass as bass
import concourse.bacc as bacc
import concourse.mybir as mybir
import concourse.tile as tile
from concourse.masks import make_identity

F32 = mybir.dt.float32
F32R = mybir.dt.float32r
BF16 = mybir.dt.bfloat16
I32 = mybir.dt.int32
I16 = mybir.dt.int16


class GATCfg:
    def __init__(self, N, E, B, Fin, layers, NC=8):
        # layers: list of dicts with H, C, concat
        self.N, self.E, self.B, self.Fin, self.NC = N, E, B, Fin, NC
        assert N % NC == 0
        self.NPC = N // NC
        self.NW = math.ceil(self.NPC / 128)
        self.NPCp = self.NW * 128
        self.layers = []
        d_in = Fin
        for l in layers:
            H, C, concat = l["H"], l["C"], l["concat"]
            d_out = H * C
            self.layers.append(
                dict(d_in=d_in, H=H, C=C, d_out=d_out, concat=concat,
                     R=d_out + 2 * H, db=(d_out if concat else C), ROW=d_out + 2 * H)
            )
            d_in = d_out if concat else C


REAL_CFG = GATCfg(
    N=50000, E=400000, B=64, Fin=128,
    layers=[dict(H=4, C=16, concat=True),
            dict(H=4, C=64, concat=True),
            dict(H=4, C=64, concat=False)],
)


# ---------------------------------------------------------------- host prep
def _host_prep(cfg, x, edge_index, batch, Ws, As, Ad, Bs):
    """Returns (in_maps, T_w). Ws/As/Ad/Bs: per-layer weight lists."""
    N, NC, NPC, NPCp, NW = cfg.N, cfg.NC, cfg.NPC, cfg.NPCp, cfg.NW
    src = np.concatenate([edge_index[0], np.arange(N, dtype=np.int64)])
    dst = np.concatenate([edge_index[1], np.arange(N, dtype=np.int64)])
    core_of = dst // NPC

    src_pad = (src // NPC) * NPCp + src % NPC
    win_global = (dst % NPC) // 128 + core_of * NW
    cnts = np.bincount(win_global, minlength=NC * NW)
    T_w = int(np.ceil(cnts.max() / 128))

    per_core = []
    for c in range(NC):
        sel = np.nonzero(core_of == c)[0]
        dloc = (dst[sel] - c * NPC).astype(np.int64)
        sp = src_pad[sel]
        win = dloc // 128
        order = np.argsort(win, kind="stable")
        sel, dloc, sp, win = sel[order], dloc[order], sp[order], win[order]
        wstart = np.searchsorted(win, np.arange(NW))
        slot = np.arange(len(sel)) - wstart[win]
        jj, pp = slot // 128, slot % 128

        esrc = np.zeros((NW, 128, T_w), np.int32)
        edrel = np.full((NW, 128, T_w), -1.0, np.float32)
        esrc[win, pp, jj] = sp.astype(np.int32)
        edrel[win, pp, jj] = (dloc - win * 128).astype(np.float32)
        # host-built dst one-hot: sdst[w, v, j, e] = (dst_rel of slot (j,e) == v)
        import ml_dtypes
        sdst = (edrel.transpose(0, 2, 1)[:, None, :, :] ==
                np.arange(128, dtype=np.float32)[None, :, None, None]
                ).astype(ml_dtypes.bfloat16)

        batchf = np.full((NW, 128, 1), -1.0, np.float32)
        bloc = batch[c * NPC:(c + 1) * NPC].astype(np.float32)
        bf = np.full(NPCp, -1.0, np.float32)
        bf[:NPC] = bloc
        batchf[:, :, 0] = bf.reshape(NW, 128)

        xT = np.zeros((cfg.Fin, NPCp), np.float32)
        xT[:, :NPC] = x[c * NPC:(c + 1) * NPC].T

        m = dict(xT=xT, esrc=esrc, sdst=sdst, edrel=edrel, batchf=batchf)
        for li, (W, a_s, a_d) in enumerate(zip(Ws, As, Ad)):
            L = cfg.layers[li]
            H, C, d_in, d_out = L["H"], L["C"], L["d_in"], L["d_out"]
            Wr = W.reshape(d_in, H, C)
            Wts = np.einsum("khc,hc->kh", Wr, a_s).astype(np.float32)
            Wtd = np.einsum("khc,hc->kh", Wr, a_d).astype(np.float32)
            m[f"waug{li}"] = np.concatenate([W, Wts, Wtd], axis=1).astype(np.float32)
            m[f"bias{li}"] = np.broadcast_to(Bs[li], (128, L["db"])).astype(np.float32).copy()
        per_core.append(m)
    return per_core, T_w


# ---------------------------------------------------------------- program
def _build_program(cfg, T_w):
    NC, NPCp, NW, B = cfg.NC, cfg.NPCp, cfg.NW, cfg.B
    NL = len(cfg.layers)
    nc = bacc.Bacc("TRN2", target_bir_lowering=False, debug=False,
                   enable_asserts=False, num_devices=cfg.NC)

    # ---- I/O
    xT_p = nc.declare_dram_parameter("xT", [cfg.Fin, NPCp], F32, isOutput=False)
    esrc_p = nc.declare_dram_parameter("esrc", [NW, 128, T_w], I32, isOutput=False)
    sdst_p = nc.declare_dram_parameter("sdst", [NW, 128, T_w, 128], BF16, isOutput=False)
    edrel_p = nc.declare_dram_parameter("edrel", [NW, 128, T_w], F32, isOutput=False)
    batchf_p = nc.declare_dram_parameter("batchf", [NW, 128, 1], F32, isOutput=False)
    waug_p, bias_p = [], []
    for li, L in enumerate(cfg.layers):
        waug_p.append(nc.declare_dram_parameter(f"waug{li}", [L["d_in"], L["R"]], F32, isOutput=False))
        bias_p.append(nc.declare_dram_parameter(f"bias{li}", [128, L["db"]], F32, isOutput=False))
    out_p = nc.declare_dram_parameter("out", [B, cfg.layers[-1]["C"]], F32, isOutput=True)

    # ---- internal DRAM
    tabloc = [nc.dram_tensor(f"tabloc{li}", [NPCp, L["ROW"]], BF16)
              for li, L in enumerate(cfg.layers)]
    tabfull = [nc.dram_tensor(f"tabfull{li}", [NC * NPCp, L["ROW"]], BF16, addr_space="Shared")
               for li, L in enumerate(cfg.layers)]

    hT = [None]
    for li in range(1, NL):
        hT.append(nc.dram_tensor(f"hT{li}", [cfg.layers[li]["d_in"], NPCp], F32))
    poolpart = nc.dram_tensor("poolpart", [B, cfg.layers[-1]["C"] + 1], F32)
    poolsum = nc.dram_tensor("poolsum", [B, cfg.layers[-1]["C"] + 1], F32, addr_space="Shared")

    rg = [list(range(NC))]

    with tile.TileContext(nc) as tc:
        with (
            tc.tile_pool(name="const", bufs=1) as constp,
            tc.tile_pool(name="wts", bufs=1) as wtsp,
            tc.tile_pool(name="proj", bufs=3) as projp,
            tc.tile_pool(name="edge", bufs=2) as edgep,
            tc.tile_pool(name="fin", bufs=2) as finp,
            tc.tile_pool(name="psmm", bufs=2, space="PSUM") as psmm,
            tc.tile_pool(name="pswin", bufs=2, space="PSUM") as pswin,
            tc.tile_pool(name="pstr", bufs=1, space="PSUM") as pstr,
            tc.tile_pool(name="psad", bufs=2, space="PSUM") as psad,
            tc.tile_pool(name="pspool", bufs=1, space="PSUM") as pspool,
        ):
            # constants
            iota_f = constp.tile([128, 128], F32)
            nc.gpsimd.iota(iota_f[:], pattern=[[1, 128]], base=0,
                           channel_multiplier=0, allow_small_or_imprecise_dtypes=True)
            ident = constp.tile([128, 128], F32)
            make_identity(nc, ident[:])
            ones = constp.tile([128, 1], F32)
            nc.vector.memset(ones[:], 1.0)

            # weights / biases resident in SBUF
            waug_sb, bias_sb = [], []
            for li, L in enumerate(cfg.layers):
                chunks = []
                d_in = L["d_in"]
                for k in range(0, d_in, 128):
                    kc = min(128, d_in - k)
                    wt = wtsp.tile([kc, L["R"]], F32, tag=f"w{li}_{k}")
                    nc.sync.dma_start(out=wt[:], in_=waug_p[li][k:k + kc, :])
                    chunks.append(wt)
                waug_sb.append(chunks)
                bt = wtsp.tile([128, L["db"]], F32, tag=f"b{li}")
                nc.sync.dma_start(out=bt[:], in_=bias_p[li][:, :])
                bias_sb.append(bt)

            pool_ps = pspool.tile([B, cfg.layers[-1]["C"] + 1], F32)

            for li, L in enumerate(cfg.layers):
                d_in, d_out, H, C, R = L["d_in"], L["d_out"], L["H"], L["C"], L["R"]
                R2 = d_out + H
                concat = L["concat"]

                # ---------------- phase A: projection + table + AllGather
                for m in range(NW):
                    ps = psmm.tile([128, R], F32)
                    nk = (d_in + 127) // 128
                    for ki, k in enumerate(range(0, d_in, 128)):
                        kc = min(128, d_in - k)
                        lh = projp.tile([kc, 128], F32, tag="lh")
                        if li == 0:
                            nc.sync.dma_start(out=lh[:], in_=xT_p[k:k + kc, m * 128:(m + 1) * 128])
                        else:
                            nc.sync.dma_start(out=lh[:], in_=hT[li][k:k + kc, m * 128:(m + 1) * 128])
                        nc.tensor.matmul(out=ps[:], lhsT=lh[:], rhs=waug_sb[li][ki][:],
                                         start=(ki == 0), stop=(ki == nk - 1))
                    tabt = projp.tile([128, L["ROW"]], BF16, tag="tabt")
                    nc.scalar.activation(out=tabt[:], in_=ps[:],
                                         func=mybir.ActivationFunctionType.Copy)
                    nc.sync.dma_start(out=tabloc[li][m * 128:(m + 1) * 128, :], in_=tabt[:])

                nc.gpsimd.collective_compute(
                    "AllGather", mybir.AluOpType.bypass, replica_groups=rg,
                    ins=[tabloc[li][:, :]], outs=[tabfull[li][:, :]],
                )

                # ---------------- phase B: edges, one 128-node window at a time
                ROW = L["ROW"]
                for w in range(NW):
                    sidx = edgep.tile([128, T_w], I32, tag="sidx")
                    nc.sync.dma_start(out=sidx[:], in_=esrc_p[w, :, :])
                    drel = edgep.tile([128, T_w], F32, tag="drel")
                    nc.sync.dma_start(out=drel[:], in_=edrel_p[w, :, :])
                    sd = edgep.tile([128, T_w, 128], BF16, tag="sd")
                    nc.sync.dma_start(out=sd[:], in_=sdst_p[w, :, :, :])
                    adw = edgep.tile([128, H], BF16, tag="adw")
                    nc.sync.dma_start(out=adw[:],
                                      in_=tabloc[li][w * 128:(w + 1) * 128, d_out + H:])

                    G = edgep.tile([128, T_w, ROW], BF16, tag="G")
                    for j in range(T_w):
                        nc.gpsimd.indirect_dma_start(
                            out=G[:, j, :], out_offset=None, in_=tabfull[li][:, :],
                            in_offset=bass.IndirectOffsetOnAxis(ap=sidx[:, j:j + 1], axis=0),
                        )
                    # per-edge a_dst via one-hot matmuls against the window rows
                    zsb = edgep.tile([128, T_w, H], F32, tag="zsb")
                    for j in range(T_w):
                        pj = psad.tile([128, H], F32)
                        nc.tensor.matmul(out=pj[:], lhsT=sd[:, j, :], rhs=adw[:],
                                         start=True, stop=True)
                        nc.scalar.activation(out=zsb[:, j, :], in_=pj[:],
                                             func=mybir.ActivationFunctionType.Copy)

                    # S[e, v] = (dst_rel[e] == v), 0/1 in f32
                    S = edgep.tile([128, T_w, 128], BF16, tag="S")
                    nc.vector.tensor_tensor(
                        out=S[:, :, :],
                        in0=drel[:, :, None].to_broadcast([128, T_w, 128]),
                        in1=iota_f[:, None, :].to_broadcast([128, T_w, 128]),
                        op=mybir.AluOpType.is_equal,
                    )

                    # logits -> p = exp(leaky_relu(a_src[src] + a_dst[dst]))
                    z = edgep.tile([128, T_w, H], F32, tag="z")
                    nc.vector.tensor_add(out=z[:, :, :], in0=G[:, :, d_out:d_out + H],
                                         in1=zsb[:, :, :])
                    zs = edgep.tile([128, T_w, H], F32, tag="zs")
                    nc.scalar.activation(out=zs[:, :, :], in_=z[:, :, :],
                                         func=mybir.ActivationFunctionType.Copy, scale=0.2)
                    zm = edgep.tile([128, T_w, H], F32, tag="zm")
                    nc.vector.tensor_max(out=zm[:, :, :], in0=z[:, :, :], in1=zs[:, :, :])
                    MT = edgep.tile([128, T_w, R2], BF16, tag="MT")
                    pf = edgep.tile([128, T_w, H], F32, tag="pf")
                    nc.scalar.activation(out=pf[:, :, :], in_=zm[:, :, :],
                                         func=mybir.ActivationFunctionType.Exp)
                    nc.vector.tensor_copy(out=MT[:, :, d_out:], in_=pf[:, :, :])
                    # M[e, h*C:(h+1)C] = p[e,h] * xp[src_e, h, :]
                    for h in range(H):
                        nc.vector.tensor_mul(
                            out=MT[:, :, h * C:(h + 1) * C],
                            in0=G[:, :, h * C:(h + 1) * C],
                            in1=MT[:, :, d_out + h:d_out + h + 1].to_broadcast([128, T_w, C]),
                        )

                    ps_w = pswin.tile([128, R2], F32)
                    for j in range(T_w):
                        nc.tensor.matmul(out=ps_w[:], lhsT=S[:, j, :], rhs=MT[:, j, :],
                                         start=(j == 0), stop=(j == T_w - 1))

                    # normalize: attn[:, hC:(h+1)C] = num / (den + eps)
                    den = finp.tile([128, H], F32, tag="den")
                    nc.vector.tensor_scalar_add(out=den[:], in0=ps_w[:, d_out:], scalar1=1e-16)
                    rcp = finp.tile([128, H], F32, tag="rcp")
                    nc.vector.reciprocal(out=rcp[:], in_=den[:])
                    attn = finp.tile([128, d_out], F32, tag="attn")
                    for h in range(H):
                        nc.scalar.activation(out=attn[:, h * C:(h + 1) * C],
                                             in_=ps_w[:, h * C:(h + 1) * C],
                                             func=mybir.ActivationFunctionType.Copy,
                                             scale=rcp[:, h:h + 1])

                    hn = finp.tile([128, L["db"] + (0 if concat else 1)], F32, tag="hn")
                    if concat:
                        hp = finp.tile([128, d_out], F32, tag="hp")
                        nc.vector.tensor_add(out=hp[:], in0=attn[:], in1=bias_sb[li][:])
                        nc.scalar.activation(out=hn[:], in_=hp[:],
                                             func=mybir.ActivationFunctionType.Gelu)
                    else:
                        hm = finp.tile([128, C], F32, tag="hm")
                        nc.vector.tensor_add(out=hm[:], in0=attn[:, 0:C], in1=attn[:, C:2 * C])
                        for h in range(2, H):
                            nc.vector.tensor_add(out=hm[:], in0=hm[:], in1=attn[:, h * C:(h + 1) * C])
                        hb = finp.tile([128, C], F32, tag="hb")
                        nc.vector.tensor_scalar(out=hb[:], in0=hm[:], scalar1=1.0 / H,
                                                scalar2=None, op0=mybir.AluOpType.mult)
                        hp2 = finp.tile([128, C], F32, tag="hp2")
                        nc.vector.tensor_add(out=hp2[:], in0=hb[:], in1=bias_sb[li][:])
                        nc.scalar.activation(out=hn[:, :C], in_=hp2[:],
                                             func=mybir.ActivationFunctionType.Gelu)
                        nc.vector.memset(hn[:, C:], 1.0)

                    if li < NL - 1:
                        # store h^T for the next projection
                        dn = L["db"]
                        for k in range(0, dn, 128):
                            kc = min(128, dn - k)
                            pt = pstr.tile([kc, 128], F32, tag="pt")
                            nc.tensor.transpose(out=pt[:], in_=hn[:, k:k + kc], identity=ident[:])
                            ht_sb = finp.tile([kc, 128], F32, tag="htsb")
                            nc.scalar.activation(out=ht_sb[:], in_=pt[:],
                                                 func=mybir.ActivationFunctionType.Copy)
                            nc.sync.dma_start(out=hT[li + 1][k:k + kc, w * 128:(w + 1) * 128],
                                              in_=ht_sb[:])
                    else:
                        # global mean pool partials: one-hot(batch) matmuls
                        bf = edgep.tile([128, 1], F32, tag="bf")
                        nc.sync.dma_start(out=bf[:], in_=batchf_p[w, :, :])
                        bsel = finp.tile([128, B], F32, tag="bsel")
                        nc.vector.tensor_tensor(
                            out=bsel[:], in0=bf[:, :1].to_broadcast([128, B]),
                            in1=iota_f[:, :B], op=mybir.AluOpType.is_equal,
                        )
                        nc.tensor.matmul(out=pool_ps[:], lhsT=bsel[:], rhs=hn[:],
                                         start=(w == 0), stop=(w == NW - 1))

            # ---------------- final pooling: AllReduce partials, divide
            C = cfg.layers[-1]["C"]
            pps = finp.tile([B, C + 1], F32, tag="pps")
            nc.scalar.activation(out=pps[:], in_=pool_ps[:],
                                 func=mybir.ActivationFunctionType.Copy)
            nc.sync.dma_start(out=poolpart[:, :], in_=pps[:])
            nc.gpsimd.collective_compute(
                "AllReduce", mybir.AluOpType.add, replica_groups=rg,
                ins=[poolpart[:, :]], outs=[poolsum[:, :]],
            )
            pl = finp.tile([B, C + 1], F32, tag="pl")
            nc.sync.dma_start(out=pl[:], in_=poolsum[:, :])
            cnt = finp.tile([B, 1], F32, tag="cnt")
            nc.vector.tensor_scalar_max(out=cnt[:], in0=pl[:, C:C + 1], scalar1=1.0)
            rc = finp.tile([B, 1], F32, tag="rc")
            nc.vector.reciprocal(out=rc[:], in_=cnt[:])
            om = finp.tile([B, C], F32, tag="om")
            nc.vector.tensor_mul(out=om[:], in0=pl[:, :C],
                                 in1=rc[:, :1].to_broadcast([B, C]))
            nc.sync.dma_start(out=out_p[:, :], in_=om[:])

    nc.finalize()
    return nc


# ---------------------------------------------------------------- entry
def _prep_and_build(cfg, x, edge_index, batch, Ws, As, Ad, Bs):
    in_maps, T_w = _host_prep(cfg, np.asarray(x), np.asarray(edge_index),
                              np.asarray(batch), Ws, As, Ad, Bs)
    nc = _build_program(cfg, T_w)
    return nc, in_maps


def kernel(x, edge_index, batch, W0, as0, ad0, b0, W1, as1, ad1, b1, W2, as2, ad2, b2):
    from concourse.bass_utils import run_bass_kernel_spmd

    cfg = REAL_CFG
    nc, in_maps = _prep_and_build(
        cfg, x, edge_index, batch,
        [np.asarray(W0), np.asarray(W1), np.asarray(W2)],
        [np.asarray(as0), np.asarray(as1), np.asarray(as2)],
        [np.asarray(ad0), np.asarray(ad1), np.asarray(ad2)],
        [np.asarray(b0), np.asarray(b1), np.asarray(b2)],
    )
    res = run_bass_kernel_spmd(nc, in_maps, list(range(cfg.NC)))
    return np.asarray(res.results[0]["out"], dtype=np.float32)


# revision 15
# speedup vs baseline: 1.8134x; 1.0150x over previous
"""GAT (3-layer, PyG-style) forward on 8 Trainium2 NeuronCores via Bass/Tile.

Strategy (dst-partitioned edges + AllGathered projection table):
  - Nodes are split into 8 contiguous shards (6250 each). Each core owns the
    edges whose *destination* lies in its shard (plus self loops), sorted by
    destination.
  - Per layer: each core projects its node shard (h @ [W | W~src | W~dst]) so
    every table row is [xp (d_out) | a_src (H) | a_dst (H)]; shards are
    AllGathered so each core holds the full projection table in local HBM.
  - Edge phase: edges are grouped by 128-node destination windows, padded to a
    uniform number of 128-edge tiles per window (uniform across cores: SPMD
    needs one program). Per window: one indirect DMA gathers all source rows,
    per-edge logits/softmax numerators are computed on DVE/ACT, and a 0/1
    selection matrix S[e,v] = (dst_e == v) turns the segment scatter-add into
    PE matmuls accumulating in PSUM (numerator and denominator together).
  - Softmax uses exp without max subtraction (logits are O(1) here; exact same
    math as the reference up to fp rounding).
  - Layer output windows are normalized, biased, GELU'd, transposed (PE) and
    written back as h^T for the next layer's projection.
  - After layer 3: global mean pool via one-hot(batch) matmuls accumulated in
    PSUM over windows, AllReduce of [64, 65] partials, divide, done.
"""

import math
import numpy as np

import concourse.bass as bass
import concourse.bacc as bacc
import concourse.mybir as mybir
import concourse.tile as tile
from concourse.masks import make_identity

F32 = mybir.dt.float32
F32R = mybir.dt.float32r
BF16 = mybir.dt.bfloat16
I32 = mybir.dt.int32
I16 = mybir.dt.int16


class GATCfg:
    def __init__(self, N, E, B, Fin, layers, NC=8):
        # layers: list of dicts with H, C, concat
        self.N, self.E, self.B, self.Fin, self.NC = N, E, B, Fin, NC
        assert N % NC == 0
        self.NPC = N // NC
        self.NW = math.ceil(self.NPC / 128)
        self.NPCp = self.NW * 128
        self.layers = []
        d_in = Fin
        for l in layers:
            H, C, concat = l["H"], l["C"], l["concat"]
            d_out = H * C
            self.layers.append(
                dict(d_in=d_in, H=H, C=C, d_out=d_out, concat=concat,
                     R=d_out + 2 * H, db=(d_out if concat else C), ROW=d_out + 2 * H)
            )
            d_in = d_out if concat else C


REAL_CFG = GATCfg(
    N=50000, E=400000, B=64, Fin=128,
    layers=[dict(H=4, C=16, concat=True),
            dict(H=4, C=64, concat=True),
            dict(H=4, C=64, concat=False)],
)


# ---------------------------------------------------------------- host prep
def _host_prep(cfg, x, edge_index, batch, Ws, As, Ad, Bs):
    """Returns (in_maps, T_w). Ws/As/Ad/Bs: per-layer weight lists."""
    N, NC, NPC, NPCp, NW = cfg.N, cfg.NC, cfg.NPC, cfg.NPCp, cfg.NW
    src = np.concatenate([edge_index[0], np.arange(N, dtype=np.int64)])
    dst = np.concatenate([edge_index[1], np.arange(N, dtype=np.int64)])
    core_of = dst // NPC

    src_pad = (src // NPC) * NPCp + src % NPC
    win_global = (dst % NPC) // 128 + core_of * NW
    cnts = np.bincount(win_global, minlength=NC * NW).reshape(NC, NW)
    tw_list = [max(1, int(np.ceil(cnts[:, w].max() / 128))) for w in range(NW)]
    off = np.concatenate([[0], np.cumsum(tw_list)]).astype(int)
    TOT = int(off[-1])

    per_core = []
    for c in range(NC):
        sel = np.nonzero(core_of == c)[0]
        dloc = (dst[sel] - c * NPC).astype(np.int64)
        sp = src_pad[sel]
        win = dloc // 128
        order = np.argsort(win, kind="stable")
        sel, dloc, sp, win = sel[order], dloc[order], sp[order], win[order]
        wstart = np.searchsorted(win, np.arange(NW))
        slot = np.arange(len(sel)) - wstart[win]
        jj, pp = slot // 128, slot % 128

        import ml_dtypes
        tidx = off[win] + jj  # global tile column
        esrc = np.zeros((128, TOT), np.int32)
        edrel = np.full((128, TOT), -1.0, np.float32)
        esrc[pp, tidx] = sp.astype(np.int32)
        edrel[pp, tidx] = (dloc - win * 128).astype(np.float32)
        # host-built dst one-hot: sdst[v, t, e] = (dst_rel of slot (t,e) == v)
        sdst = (edrel.T[None, :, :] ==
                np.arange(128, dtype=np.float32)[:, None, None]
                ).astype(ml_dtypes.bfloat16)

        batchf = np.full((NW, 128, 1), -1.0, np.float32)
        bloc = batch[c * NPC:(c + 1) * NPC].astype(np.float32)
        bf = np.full(NPCp, -1.0, np.float32)
        bf[:NPC] = bloc
        batchf[:, :, 0] = bf.reshape(NW, 128)

        xT = np.zeros((cfg.Fin, NPCp), np.float32)
        xT[:, :NPC] = x[c * NPC:(c + 1) * NPC].T

        m = dict(xT=xT, esrc=esrc, sdst=sdst, edrel=edrel, batchf=batchf)
        for li, (W, a_s, a_d) in enumerate(zip(Ws, As, Ad)):
            L = cfg.layers[li]
            H, C, d_in, d_out = L["H"], L["C"], L["d_in"], L["d_out"]
            Wr = W.reshape(d_in, H, C)
            Wts = np.einsum("khc,hc->kh", Wr, a_s).astype(np.float32)
            Wtd = np.einsum("khc,hc->kh", Wr, a_d).astype(np.float32)
            m[f"waug{li}"] = np.concatenate([W, Wts, Wtd], axis=1).astype(np.float32)
            m[f"bias{li}"] = np.broadcast_to(Bs[li], (128, L["db"])).astype(np.float32).copy()
        per_core.append(m)
    return per_core, (tw_list, off, TOT)


# ---------------------------------------------------------------- program
def _build_program(cfg, tws):
    tw_list, off, TOT = tws
    NC, NPCp, NW, B = cfg.NC, cfg.NPCp, cfg.NW, cfg.B
    NL = len(cfg.layers)
    nc = bacc.Bacc("TRN2", target_bir_lowering=False, debug=False,
                   enable_asserts=False, num_devices=cfg.NC)

    # ---- I/O
    xT_p = nc.declare_dram_parameter("xT", [cfg.Fin, NPCp], F32, isOutput=False)
    esrc_p = nc.declare_dram_parameter("esrc", [128, TOT], I32, isOutput=False)
    sdst_p = nc.declare_dram_parameter("sdst", [128, TOT, 128], BF16, isOutput=False)
    edrel_p = nc.declare_dram_parameter("edrel", [128, TOT], F32, isOutput=False)
    batchf_p = nc.declare_dram_parameter("batchf", [NW, 128, 1], F32, isOutput=False)
    waug_p, bias_p = [], []
    for li, L in enumerate(cfg.layers):
        waug_p.append(nc.declare_dram_parameter(f"waug{li}", [L["d_in"], L["R"]], F32, isOutput=False))
        bias_p.append(nc.declare_dram_parameter(f"bias{li}", [128, L["db"]], F32, isOutput=False))
    out_p = nc.declare_dram_parameter("out", [B, cfg.layers[-1]["C"]], F32, isOutput=True)

    # ---- internal DRAM
    tabloc = [nc.dram_tensor(f"tabloc{li}", [NPCp, L["ROW"]], BF16)
              for li, L in enumerate(cfg.layers)]
    tabfull = [nc.dram_tensor(f"tabfull{li}", [NC * NPCp, L["ROW"]], BF16, addr_space="Shared")
               for li, L in enumerate(cfg.layers)]

    poolpart = nc.dram_tensor("poolpart", [B, cfg.layers[-1]["C"] + 1], F32)
    poolsum = nc.dram_tensor("poolsum", [B, cfg.layers[-1]["C"] + 1], F32, addr_space="Shared")

    rg = [list(range(NC))]

    with tile.TileContext(nc) as tc:
        with (
            tc.tile_pool(name="const", bufs=1) as constp,
            tc.tile_pool(name="wts", bufs=1) as wtsp,
            tc.tile_pool(name="proj", bufs=3) as projp,
            tc.tile_pool(name="edge", bufs=2) as edgep,
            tc.tile_pool(name="fin", bufs=2) as finp,
            tc.tile_pool(name="psmm", bufs=2, space="PSUM") as psmm,
            tc.tile_pool(name="pswin", bufs=2, space="PSUM") as pswin,
            tc.tile_pool(name="pstr", bufs=1, space="PSUM") as pstr,
            tc.tile_pool(name="psad", bufs=2, space="PSUM") as psad,
            tc.tile_pool(name="pspool", bufs=1, space="PSUM") as pspool,
        ):
            # constants
            iota_f = constp.tile([128, 128], F32)
            nc.gpsimd.iota(iota_f[:], pattern=[[1, 128]], base=0,
                           channel_multiplier=0, allow_small_or_imprecise_dtypes=True)
            ident = constp.tile([128, 128], F32)
            make_identity(nc, ident[:])
            ones = constp.tile([128, 1], F32)
            nc.vector.memset(ones[:], 1.0)

            # weights / biases resident in SBUF
            waug_sb, bias_sb = [], []
            for li, L in enumerate(cfg.layers):
                chunks = []
                d_in = L["d_in"]
                for k in range(0, d_in, 128):
                    kc = min(128, d_in - k)
                    wt = wtsp.tile([kc, L["R"]], F32, tag=f"w{li}_{k}")
                    nc.sync.dma_start(out=wt[:], in_=waug_p[li][k:k + kc, :])
                    chunks.append(wt)
                waug_sb.append(chunks)
                bt = wtsp.tile([128, L["db"]], F32, tag=f"b{li}")
                nc.sync.dma_start(out=bt[:], in_=bias_p[li][:, :])
                bias_sb.append(bt)

            pool_ps = pspool.tile([B, cfg.layers[-1]["C"] + 1], F32)

            # ---------------- layer-0 projection prologue (input is replicated)
            L0 = cfg.layers[0]
            for m in range(NW):
                ps = psmm.tile([128, L0["R"]], F32, tag="ps")
                nk = (L0["d_in"] + 127) // 128
                for ki, k in enumerate(range(0, L0["d_in"], 128)):
                    kc = min(128, L0["d_in"] - k)
                    lh = projp.tile([kc, 128], F32, tag="lh")
                    nc.sync.dma_start(out=lh[:], in_=xT_p[k:k + kc, m * 128:(m + 1) * 128])
                    nc.tensor.matmul(out=ps[:], lhsT=lh[:], rhs=waug_sb[0][ki][:],
                                     start=(ki == 0), stop=(ki == nk - 1))
                tabt = projp.tile([128, L0["ROW"]], BF16, tag="tabt")
                nc.scalar.activation(out=tabt[:], in_=ps[:],
                                     func=mybir.ActivationFunctionType.Copy)
                nc.sync.dma_start(out=tabloc[0][m * 128:(m + 1) * 128, :], in_=tabt[:])

            for li, L in enumerate(cfg.layers):
                d_in, d_out, H, C, R = L["d_in"], L["d_out"], L["H"], L["C"], L["R"]
                R2 = d_out + H
                concat = L["concat"]

                nc.gpsimd.collective_compute(
                    "AllGather", mybir.AluOpType.bypass, replica_groups=rg,
                    ins=[tabloc[li][:, :]], outs=[tabfull[li][:, :]],
                )

                # ---------------- phase B: edges, one 128-node window at a time
                ROW = L["ROW"]
                for w in range(NW):
                    T_w = tw_list[w]
                    o0, o1 = int(off[w]), int(off[w + 1])
                    sidx = edgep.tile([128, T_w], I32, tag="sidx")
                    nc.sync.dma_start(out=sidx[:], in_=esrc_p[:, o0:o1])
                    drel = edgep.tile([128, T_w], F32, tag="drel")
                    nc.sync.dma_start(out=drel[:], in_=edrel_p[:, o0:o1])
                    sd = edgep.tile([128, T_w, 128], BF16, tag="sd")
                    nc.sync.dma_start(out=sd[:], in_=sdst_p[:, o0:o1, :])
                    adw = edgep.tile([128, H], BF16, tag="adw")
                    nc.sync.dma_start(out=adw[:],
                                      in_=tabloc[li][w * 128:(w + 1) * 128, d_out + H:])

                    G = edgep.tile([128, T_w, ROW], BF16, tag="G")
                    for j in range(T_w):
                        nc.gpsimd.indirect_dma_start(
                            out=G[:, j, :], out_offset=None, in_=tabfull[li][:, :],
                            in_offset=bass.IndirectOffsetOnAxis(ap=sidx[:, j:j + 1], axis=0),
                        )
                    # per-edge a_dst via one-hot matmuls against the window rows
                    zsb = edgep.tile([128, T_w, H], F32, tag="zsb")
                    for j in range(T_w):
                        pj = psad.tile([128, H], F32)
                        nc.tensor.matmul(out=pj[:], lhsT=sd[:, j, :], rhs=adw[:],
                                         start=True, stop=True)
                        nc.scalar.activation(out=zsb[:, j, :], in_=pj[:],
                                             func=mybir.ActivationFunctionType.Copy)

                    # S[e, v] = (dst_rel[e] == v), 0/1 in f32
                    S = edgep.tile([128, T_w, 128], BF16, tag="S")
                    nc.vector.tensor_tensor(
                        out=S[:, :, :],
                        in0=drel[:, :, None].to_broadcast([128, T_w, 128]),
                        in1=iota_f[:, None, :].to_broadcast([128, T_w, 128]),
                        op=mybir.AluOpType.is_equal,
                    )

                    # logits -> p = exp(leaky_relu(a_src[src] + a_dst[dst]))
                    z = edgep.tile([128, T_w, H], F32, tag="z")
                    nc.vector.tensor_add(out=z[:, :, :], in0=G[:, :, d_out:d_out + H],
                                         in1=zsb[:, :, :])
                    zs = edgep.tile([128, T_w, H], F32, tag="zs")
                    nc.scalar.activation(out=zs[:, :, :], in_=z[:, :, :],
                                         func=mybir.ActivationFunctionType.Copy, scale=0.2)
                    zm = edgep.tile([128, T_w, H], F32, tag="zm")
                    nc.vector.tensor_max(out=zm[:, :, :], in0=z[:, :, :], in1=zs[:, :, :])
                    MT = edgep.tile([128, T_w, R2], BF16, tag="MT")
                    pf = edgep.tile([128, T_w, H], F32, tag="pf")
                    nc.scalar.activation(out=pf[:, :, :], in_=zm[:, :, :],
                                         func=mybir.ActivationFunctionType.Exp)
                    nc.vector.tensor_copy(out=MT[:, :, d_out:], in_=pf[:, :, :])
                    # M[e, h*C:(h+1)C] = p[e,h] * xp[src_e, h, :]
                    for h in range(H):
                        nc.vector.tensor_mul(
                            out=MT[:, :, h * C:(h + 1) * C],
                            in0=G[:, :, h * C:(h + 1) * C],
                            in1=MT[:, :, d_out + h:d_out + h + 1].to_broadcast([128, T_w, C]),
                        )

                    ps_w = pswin.tile([128, R2], F32)
                    for j in range(T_w):
                        nc.tensor.matmul(out=ps_w[:], lhsT=S[:, j, :], rhs=MT[:, j, :],
                                         start=(j == 0), stop=(j == T_w - 1))

                    # normalize: attn[:, hC:(h+1)C] = num / (den + eps)
                    den = finp.tile([128, H], F32, tag="den")
                    nc.vector.tensor_scalar_add(out=den[:], in0=ps_w[:, d_out:], scalar1=1e-16)
                    rcp = finp.tile([128, H], F32, tag="rcp")
                    nc.vector.reciprocal(out=rcp[:], in_=den[:])
                    attn = finp.tile([128, d_out], F32, tag="attn")
                    for h in range(H):
                        nc.scalar.activation(out=attn[:, h * C:(h + 1) * C],
                                             in_=ps_w[:, h * C:(h + 1) * C],
                                             func=mybir.ActivationFunctionType.Copy,
                                             scale=rcp[:, h:h + 1])

                    hn = finp.tile([128, L["db"] + (0 if concat else 1)], F32, tag="hn")
                    if concat:
                        hp = finp.tile([128, d_out], F32, tag="hp")
                        nc.vector.tensor_add(out=hp[:], in0=attn[:], in1=bias_sb[li][:])
                        nc.scalar.activation(out=hn[:], in_=hp[:],
                                             func=mybir.ActivationFunctionType.Gelu)
                    else:
                        hm = finp.tile([128, C], F32, tag="hm")
                        nc.vector.tensor_add(out=hm[:], in0=attn[:, 0:C], in1=attn[:, C:2 * C])
                        for h in range(2, H):
                            nc.vector.tensor_add(out=hm[:], in0=hm[:], in1=attn[:, h * C:(h + 1) * C])
                        hb = finp.tile([128, C], F32, tag="hb")
                        nc.vector.tensor_scalar(out=hb[:], in0=hm[:], scalar1=1.0 / H,
                                                scalar2=None, op0=mybir.AluOpType.mult)
                        hp2 = finp.tile([128, C], F32, tag="hp2")
                        nc.vector.tensor_add(out=hp2[:], in0=hb[:], in1=bias_sb[li][:])
                        nc.scalar.activation(out=hn[:, :C], in_=hp2[:],
                                             func=mybir.ActivationFunctionType.Gelu)
                        nc.vector.memset(hn[:, C:], 1.0)

                    if li < NL - 1:
                        # transpose h and immediately project for the next layer
                        Ln = cfg.layers[li + 1]
                        dn = L["db"]
                        nk = (dn + 127) // 128
                        ps2 = psmm.tile([128, Ln["R"]], F32, tag="ps")
                        for ki, k in enumerate(range(0, dn, 128)):
                            kc = min(128, dn - k)
                            pt = pstr.tile([kc, 128], F32, tag="pt")
                            nc.tensor.transpose(out=pt[:], in_=hn[:, k:k + kc], identity=ident[:])
                            ht_sb = finp.tile([kc, 128], F32, tag="htsb")
                            nc.scalar.activation(out=ht_sb[:], in_=pt[:],
                                                 func=mybir.ActivationFunctionType.Copy)
                            nc.tensor.matmul(out=ps2[:], lhsT=ht_sb[:], rhs=waug_sb[li + 1][ki][:],
                                             start=(ki == 0), stop=(ki == nk - 1))
                        tabt2 = projp.tile([128, Ln["ROW"]], BF16, tag="tabt")
                        nc.scalar.activation(out=tabt2[:], in_=ps2[:],
                                             func=mybir.ActivationFunctionType.Copy)
                        nc.sync.dma_start(out=tabloc[li + 1][w * 128:(w + 1) * 128, :],
                                          in_=tabt2[:])
                    else:
                        # global mean pool partials: one-hot(batch) matmuls
                        bf = edgep.tile([128, 1], F32, tag="bf")
                        nc.sync.dma_start(out=bf[:], in_=batchf_p[w, :, :])
                        bsel = finp.tile([128, B], F32, tag="bsel")
                        nc.vector.tensor_tensor(
                            out=bsel[:], in0=bf[:, :1].to_broadcast([128, B]),
                            in1=iota_f[:, :B], op=mybir.AluOpType.is_equal,
                        )
                        nc.tensor.matmul(out=pool_ps[:], lhsT=bsel[:], rhs=hn[:],
                                         start=(w == 0), stop=(w == NW - 1))

            # ---------------- final pooling: AllReduce partials, divide
            C = cfg.layers[-1]["C"]
            pps = finp.tile([B, C + 1], F32, tag="pps")
            nc.scalar.activation(out=pps[:], in_=pool_ps[:],
                                 func=mybir.ActivationFunctionType.Copy)
            nc.sync.dma_start(out=poolpart[:, :], in_=pps[:])
            nc.gpsimd.collective_compute(
                "AllReduce", mybir.AluOpType.add, replica_groups=rg,
                ins=[poolpart[:, :]], outs=[poolsum[:, :]],
            )
            pl = finp.tile([B, C + 1], F32, tag="pl")
            nc.sync.dma_start(out=pl[:], in_=poolsum[:, :])
            cnt = finp.tile([B, 1], F32, tag="cnt")
            nc.vector.tensor_scalar_max(out=cnt[:], in0=pl[:, C:C + 1], scalar1=1.0)
            rc = finp.tile([B, 1], F32, tag="rc")
            nc.vector.reciprocal(out=rc[:], in_=cnt[:])
            om = finp.tile([B, C], F32, tag="om")
            nc.vector.tensor_mul(out=om[:], in0=pl[:, :C],
                                 in1=rc[:, :1].to_broadcast([B, C]))
            nc.sync.dma_start(out=out_p[:, :], in_=om[:])

    nc.finalize()
    return nc


# ---------------------------------------------------------------- entry
def _prep_and_build(cfg, x, edge_index, batch, Ws, As, Ad, Bs):
    in_maps, T_w = _host_prep(cfg, np.asarray(x), np.asarray(edge_index),
                              np.asarray(batch), Ws, As, Ad, Bs)
    nc = _build_program(cfg, T_w)
    return nc, in_maps


def kernel(x, edge_index, batch, W0, as0, ad0, b0, W1, as1, ad1, b1, W2, as2, ad2, b2):
    from concourse.bass_utils import run_bass_kernel_spmd

    cfg = REAL_CFG
    nc, in_maps = _prep_and_build(
        cfg, x, edge_index, batch,
        [np.asarray(W0), np.asarray(W1), np.asarray(W2)],
        [np.asarray(as0), np.asarray(as1), np.asarray(as2)],
        [np.asarray(ad0), np.asarray(ad1), np.asarray(ad2)],
        [np.asarray(b0), np.asarray(b1), np.asarray(b2)],
    )
    res = run_bass_kernel_spmd(nc, in_maps, list(range(cfg.NC)))
    return np.asarray(res.results[0]["out"], dtype=np.float32)
